# revision 30
# baseline (speedup 1.0000x reference)
"""Trainium2 Bass kernel for nn_GroupedConvFuseSide4.

out[b,k] = w[k,0]*side5[b,k] + w[k,1]*side4[b,k]
         + w[k,2]*side1[b,0] + w[k,3]*side2[b,0] + w[k,4]*side3[b,0] + bias[k]

Sharding: pure data parallel over batch (B=8) across 8 NeuronCores.

v14 scheme — fp16 wire, half-tile streaming, per-core phase rotation:
  Pixels of one image are split into CH=32 chunks of FD=8192.  Row
  r = 19*g + k gives ROWS=608 rows; 5 row-tiles (4x128 + 96) x 2
  column halves stream through SBUF as [R, 4096] 1 MB transfers with
  7-deep pools, so the free->load->use chain never starves compute.
  All 8 NeuronCores share one device's ~2.9 TB/s HBM and run the same
  SPMD program in lockstep, so every core otherwise reads the same
  relative buffer offsets at the same instant (same HBM channel
  phase).  To decorrelate, core c processes the four full row-tiles
  in a rotated order: the host row-permutes each core's packed x5/x4
  (and builds a matching per-core const blob), so the shared program
  always reads slot rows [128*s, 128*s+128) while core c's slot s
  holds physical tile (s + c) % 4.  The output is un-permuted on host.
  The three singles + a ones row are RESIDENT: one [97, FD] SBUF tile
  (row 32*s+g, row 96 = ones for bias) loaded once; the rhs column of
  the singles matmul is the pixel position within a chunk, so the
  rotation does not touch it.  Per 512-col region:
    - PE: diag(w1) @ x4  (start=True) + lsT @ [singles; ones]
      (stop=True).  Both stationary operands are 128-col slices of ONE
      per-core [128, 1232] fp16 const blob (LS | GD | f32-scalars
      bitcast), a single ~2.4 KB/partition DMA.  All 4 diag matmuls
      run back-to-back, then all 4 ls matmuls, so the PE pays 2 weight
      swaps per block instead of 8 (a swap costs ~170ns of refill).
    - DVE: one STT per [R, 2048] block: out = x5*w0 + psum (measured
      2.34us/block; a single full pass is the DVE minimum since STT
      never gets the 2x packed mode).
  A 10-matmul warmup burst runs at kernel start so the PE HAM clock
  gate flips to 2.4 GHz before the real matmuls begin.
  DMA queues balanced ~9.9 MB each: sync = blob + all x5 loads,
  scalar = all x4 loads + tail-tile 64-part stores, gpsimd = singles
  + half-tile 1 MB stores + tail-tile 32-part stores.  Loads are
  never queued behind stores on any queue.  The 96-row tail tile is
  processed mid-stream and the last tile's stores fan out across the
  by-then idle HWDGE queues.  All DMA partition counts are in
  {1, 32, 64, 128} (counts in 65..127 hit degenerate descriptor
  paths).  Host converts to fp16 and repacks so every load is
  contiguous; output comes back fp16, upcast on host.
  Max rel err vs the f32 reference ~8e-4, well under the 2e-2 gate.
"""

import numpy as np

B, K, H, W = 8, 19, 512, 512
FD = 8192                  # pixels per chunk
CH = 32                    # chunks per image (H*W / FD)
ROWS = K * CH              # 608 packed rows per core
TILES = []                 # slot (row0, nrows): 4 x 128 + 1 x 96
_r = 0
while _r < ROWS:
    TILES.append((_r, min(128, ROWS - _r)))
    _r += 128
NT = len(TILES)
HW_ = FD // 2              # half-tile columns (4096)
N_CORES = 8
BLOB_W = 1232              # 608 LS + 608 GD + 10 scl-f32-raw + 6 pad

_cache = {}


def _build_program():
    import concourse.bacc as bacc
    import concourse.tile as tile
    import concourse.mybir as mybir
    from contextlib import ExitStack

    f16 = mybir.dt.float16
    f32 = mybir.dt.float32
    mult = mybir.AluOpType.mult
    add = mybir.AluOpType.add

    nc = bacc.Bacc(
        "TRN2", target_bir_lowering=False, debug=False,
        enable_asserts=False, num_devices=N_CORES,
        enable_partition_id=False,
    )

    x5_d = nc.dram_tensor("x5", [ROWS, FD], f16, kind="ExternalInput").ap()
    x4_d = nc.dram_tensor("x4", [ROWS, FD], f16, kind="ExternalInput").ap()
    s_d = nc.dram_tensor("s", [97, FD], f16, kind="ExternalInput").ap()
    blob_d = nc.dram_tensor("blob", [128, BLOB_W], f16,
                            kind="ExternalInput").ap()
    out_d = nc.dram_tensor("out", [ROWS, FD], f16, kind="ExternalOutput").ap()

    with tile.TileContext(nc) as tc, ExitStack() as ctx:
        consts = ctx.enter_context(tc.tile_pool(name="consts", bufs=1))
        x5_pool = ctx.enter_context(tc.tile_pool(name="x5", bufs=7))
        x4_pool = ctx.enter_context(tc.tile_pool(name="x4", bufs=7))
        o_pool = ctx.enter_context(tc.tile_pool(name="o", bufs=4))
        o4_pool = ctx.enter_context(tc.tile_pool(name="o4", bufs=4))
        ps_pool = ctx.enter_context(tc.tile_pool(name="ps", bufs=2, space="PSUM"))

        blob_t = consts.tile([128, BLOB_W], f16, tag="blob")
        nc.sync.dma_start(out=blob_t[:], in_=blob_d)
        s_t = consts.tile([97, FD], f16, tag="s")
        nc.gpsimd.dma_start(out=s_t[0:64], in_=s_d[0:64])
        nc.gpsimd.dma_start(out=s_t[64:96], in_=s_d[64:96])
        nc.gpsimd.dma_start(out=s_t[96:97], in_=s_d[96:97])
        scl_t = blob_t[:, 1216:1216 + 2 * NT].bitcast(f32)

        # ---- PE warmup: dense matmuls so the HAM clock gate flips to
        # 2.4 GHz before the first real matmul; results are dead writes.
        wu = consts.tile([128, 512], f16, tag="wu")
        nc.vector.memset(wu[:], 0.0)
        ps_w = ps_pool.tile([128, 2048], f32, tag="ps", name="ps_warm")
        for i in range(10):
            nc.tensor.matmul(
                ps_w[:, 0:512], wu[:, 0:128], wu[:],
                start=True, stop=True, skip_group_check=True,
            )

        # process the fiddly 96-row tail slot mid-stream (its small split
        # DMAs ride the DMA-bound middle's slack) so the kernel ends on
        # full-width tiles, and the last tile's stores fan out across the
        # by-then idle HWDGE queues.
        order = [0, 1, 4, 2, 3] if NT == 5 else list(range(NT))
        for t in order:
            r0, R = TILES[t]
            last = t == order[-1]
            diagT = blob_t[0:R, 608 + r0:608 + r0 + R]
            lsT = blob_t[0:97, r0:r0 + R]
            w0 = scl_t[0:R, t:t + 1]
            for half in range(2):
                c0 = HW_ * half
                x5h = x5_pool.tile([R, HW_], f16, tag="x5", name=f"x5_{t}_{half}")
                x4h = x4_pool.tile([R, HW_], f16, tag="x4", name=f"x4_{t}_{half}")
                nc.sync.dma_start(out=x5h[:], in_=x5_d[r0:r0 + R, c0:c0 + HW_])
                nc.scalar.dma_start(out=x4h[:], in_=x4_d[r0:r0 + R, c0:c0 + HW_])
                oh = (o_pool.tile([R, HW_], f16, tag="o", name=f"o_{t}_{half}")
                      if R == 128 else None)

                for sub in range(2):
                    lc = 2048 * sub           # local col in half tile
                    gc = c0 + lc              # global col
                    ps = ps_pool.tile([R, 2048], f32, tag="ps",
                                      name=f"ps_{t}_{half}_{sub}")
                    for h in range(4):
                        nc.tensor.matmul(
                            ps[:, 512 * h:512 * h + 512], diagT,
                            x4h[:, lc + 512 * h:lc + 512 * h + 512],
                            start=True, stop=False, skip_group_check=True,
                        )
                    for h in range(4):
                        nc.tensor.matmul(
                            ps[:, 512 * h:512 * h + 512], lsT,
                            s_t[0:97, gc + 512 * h:gc + 512 * h + 512],
                            start=False, stop=True, skip_group_check=True,
                        )
                    if oh is not None:
                        nc.vector.scalar_tensor_tensor(
                            oh[:, lc:lc + 2048], x5h[:, lc:lc + 2048], w0,
                            ps[:], mult, add)
                    else:
                        o4 = o4_pool.tile([R, 2048], f16, tag="o4",
                                          name=f"o4_{half}_{sub}")
                        nc.vector.scalar_tensor_tensor(
                            o4[:], x5h[:, lc:lc + 2048], w0, ps[:], mult, add)
                        qe = nc.scalar if sub == 0 else nc.gpsimd
                        qe.dma_start(
                            out=out_d[r0:r0 + R, gc:gc + 2048], in_=o4[:])
                if oh is not None:
                    if last:
                        nc.sync.dma_start(
                            out=out_d[r0:r0 + 64, c0:c0 + HW_], in_=oh[0:64])
                        nc.scalar.dma_start(
                            out=out_d[r0 + 64:r0 + R, c0:c0 + HW_],
                            in_=oh[64:R])
                    else:
                        nc.gpsimd.dma_start(out=out_d[r0:r0 + R, c0:c0 + HW_],
                                            in_=oh[:])

    nc.compile()
    return nc


def _get_program():
    if "nc" not in _cache:
        _cache["nc"] = _build_program()
    return _cache["nc"]


def _rowmap(core):
    """slot row p -> physical row, rotating the four 128-row tiles by core."""
    m = np.empty(ROWS, dtype=np.int64)
    for s in range(4):
        tp = (s + core) % 4
        m[128 * s:128 * s + 128] = np.arange(128 * tp, 128 * tp + 128)
    m[512:608] = np.arange(512, 608)
    return m


def _blob_for(w, b, rowmap):
    """Per-core const blob in slot coordinates."""
    q = rowmap                      # physical row for each slot row p
    kk = q % K
    gg = q // K
    pp = np.arange(ROWS)
    blob = np.zeros((128, BLOB_W), dtype=np.float16)
    # LS[32*s + g(q), p] = w[k(q), 2+s]; LS[96, p] = bias[k(q)]
    for s in range(3):
        blob[32 * s + gg, pp] = w[kk, 2 + s].astype(np.float16)
    blob[96, pp] = b[kk].astype(np.float16)
    # GD[p % 128, 608 + p] = w1[k(q)] (diagonal in slot coords)
    blob[pp % 128, 608 + pp] = w[kk, 1].astype(np.float16)
    # w0 per-partition f32 scalars per slot, raw-bitcast into fp16 cols
    scl = np.zeros((128, NT), dtype=np.float32)
    for t, (r0, R) in enumerate(TILES):
        scl[:R, t] = w[kk[r0:r0 + R], 0]
    blob[:, 1216:1216 + 2 * NT] = scl.view(np.float16)
    return blob


def _pack_kchw(a16):
    """[K, CH, FD] fp16 -> [ROWS, FD], row = 19*g + k."""
    return np.ascontiguousarray(a16.transpose(1, 0, 2)).reshape(ROWS, FD)


def run(inputs, trace=False, tmpdir=None):
    from concourse.bass_utils import run_bass_kernel_spmd

    w = np.asarray(inputs["weight"], dtype=np.float32)
    b = np.asarray(inputs["bias"], dtype=np.float32)
    nc = _get_program()

    s1h = np.asarray(inputs["side1"]).astype(np.float16).reshape(B, CH, FD)
    s2h = np.asarray(inputs["side2"]).astype(np.float16).reshape(B, CH, FD)
    s3h = np.asarray(inputs["side3"]).astype(np.float16).reshape(B, CH, FD)
    s4h = np.asarray(inputs["side4"]).astype(np.float16).reshape(B, K, CH, FD)
    s5h = np.asarray(inputs["side5"]).astype(np.float16).reshape(B, K, CH, FD)

    in_maps = []
    rowmaps = []
    for c in range(N_CORES):
        sp = np.empty((97, FD), dtype=np.float16)
        sp[0:32] = s1h[c]
        sp[32:64] = s2h[c]
        sp[64:96] = s3h[c]
        sp[96] = np.float16(1.0)
        rm = _rowmap(c)
        rowmaps.append(rm)
        # cores 4-7 additionally swap the two column halves of every row
        # (and of the singles, so matmul columns stay consistent) to break
        # the remaining lockstep pairing with cores 0-3; the swap is its
        # own inverse and is undone on the output below.
        if c >= 4:
            sp[0:96] = np.concatenate([sp[0:96, HW_:], sp[0:96, :HW_]], axis=1)
            x5p = _pack_kchw(s5h[c])[rm]
            x4p = _pack_kchw(s4h[c])[rm]
            x5p = np.concatenate([x5p[:, HW_:], x5p[:, :HW_]], axis=1)
            x4p = np.concatenate([x4p[:, HW_:], x4p[:, :HW_]], axis=1)
        else:
            x5p = _pack_kchw(s5h[c])[rm]
            x4p = _pack_kchw(s4h[c])[rm]
        in_maps.append({
            "x5": x5p,
            "x4": x4p,
            "s": sp,
            "blob": _blob_for(w, b, rm),
        })

    res = run_bass_kernel_spmd(nc, in_maps, list(range(N_CORES)),
                               trace=trace, tmpdir=tmpdir)
    outs = []
    for c in range(N_CORES):
        o_slot = res.results[c]["out"]
        if c >= 4:
            o_slot = np.concatenate([o_slot[:, HW_:], o_slot[:, :HW_]], axis=1)
        o_phys = np.empty_like(o_slot)
        o_phys[rowmaps[c]] = o_slot
        o = o_phys.reshape(CH, K, FD).transpose(1, 0, 2)
        outs.append(o.reshape(1, K, H, W).astype(np.float32))
    return np.concatenate(outs, axis=0), res


def kernel(**inputs):
    out, _ = run(inputs, trace=False)
    return out


# revision 31
# speedup vs baseline: 1.0377x; 1.0377x over previous
"""Trainium2 Bass kernel for nn_GroupedConvFuseSide4.

out[b,k] = w[k,0]*side5[b,k] + w[k,1]*side4[b,k]
         + w[k,2]*side1[b,0] + w[k,3]*side2[b,0] + w[k,4]*side3[b,0] + bias[k]

Sharding: pure data parallel over batch (B=8) across 8 NeuronCores.

Final scheme — fp16 wire, half-tile streaming, per-core phase rotation:
  Pixels of one image are split into CH=32 chunks of FD=8192.  Row
  r = 19*g + k gives ROWS=608 rows; 5 row-tiles (4x128 + 96) x 2
  column halves stream through SBUF as [R, 4096] 1 MB transfers with
  7-deep pools, so the free->load->use chain never starves compute.
  All 8 NeuronCores share one device's ~2.9 TB/s HBM and run the same
  SPMD program in lockstep, so every core otherwise reads the same
  relative buffer offsets at the same instant (same HBM channel
  phase).  To decorrelate, core c processes the four full row-tiles
  in a rotated order: the host row-permutes each core's packed x5/x4
  (and builds a matching per-core const blob), so the shared program
  always reads slot rows [128*s, 128*s+128) while core c's slot s
  holds physical tile (s + c) % 4.  The output is un-permuted on host.
  The three singles + a ones row are RESIDENT: one [97, FD] SBUF tile
  (row 32*s+g, row 96 = ones for bias) loaded once; the rhs column of
  the singles matmul is the pixel position within a chunk, so the
  rotation does not touch it.  Per 512-col region:
    - PE: diag(w1) @ x4  (start=True) + lsT @ [singles; ones]
      (stop=True).  Both stationary operands are 128-col slices of ONE
      per-core [128, 1232] fp16 const blob (LS | GD | f32-scalars
      bitcast), a single ~2.4 KB/partition DMA.  All 4 diag matmuls
      run back-to-back, then all 4 ls matmuls, so the PE pays 2 weight
      swaps per block instead of 8 (a swap costs ~170ns of refill).
    - DVE: one STT per [R, 2048] block: out = x5*w0 + psum (measured
      2.34us/block; a single full pass is the DVE minimum since STT
      never gets the 2x packed mode).
  A 10-matmul warmup burst runs at kernel start so the PE HAM clock
  gate flips to 2.4 GHz before the real matmuls begin.
  Cores 4-7 additionally swap the two column halves of every row (and
  of the singles) so all 8 cores have distinct (rotation, half) phase
  offsets.  DMA queues balanced ~9.9 MB each: sync = blob + all x5
  loads, scalar = all x4 loads, gpsimd = singles + half-tile 1 MB
  stores; the 96-row tail tile's block stores alternate scalar/gpsimd
  and the last tile's stores fan out across the by-then idle HWDGE
  queues.  Loads are never queued behind stores on any queue.
  (Direct [96, N] DMAs measured FASTER than 64+32 splits — the v3-era
  "counts >64 and !=128 are degenerate" rule did not reproduce.)
  Host converts to fp16 and repacks so every load is contiguous;
  output comes back fp16, upcast on host.
  Max rel err vs the f32 reference ~8e-4, well under the 2e-2 gate.
"""

import numpy as np

B, K, H, W = 8, 19, 512, 512
FD = 8192                  # pixels per chunk
CH = 32                    # chunks per image (H*W / FD)
ROWS = K * CH              # 608 packed rows per core
TILES = []                 # slot (row0, nrows): 4 x 128 + 1 x 96
_r = 0
while _r < ROWS:
    TILES.append((_r, min(128, ROWS - _r)))
    _r += 128
NT = len(TILES)
HW_ = FD // 2              # half-tile columns (4096)
N_CORES = 8
BLOB_W = 1232              # 608 LS + 608 GD + 10 scl-f32-raw + 6 pad

_cache = {}


def _build_program():
    import concourse.bacc as bacc
    import concourse.tile as tile
    import concourse.mybir as mybir
    from contextlib import ExitStack

    f16 = mybir.dt.float16
    f32 = mybir.dt.float32
    mult = mybir.AluOpType.mult
    add = mybir.AluOpType.add

    nc = bacc.Bacc(
        "TRN2", target_bir_lowering=False, debug=False,
        enable_asserts=False, num_devices=N_CORES,
        enable_partition_id=False,
    )

    x5_d = nc.dram_tensor("x5", [ROWS, FD], f16, kind="ExternalInput").ap()
    x4_d = nc.dram_tensor("x4", [ROWS, FD], f16, kind="ExternalInput").ap()
    s_d = nc.dram_tensor("s", [97, FD], f16, kind="ExternalInput").ap()
    blob_d = nc.dram_tensor("blob", [128, BLOB_W], f16,
                            kind="ExternalInput").ap()
    out_d = nc.dram_tensor("out", [ROWS, FD], f16, kind="ExternalOutput").ap()

    with tile.TileContext(nc) as tc, ExitStack() as ctx:
        consts = ctx.enter_context(tc.tile_pool(name="consts", bufs=1))
        x5_pool = ctx.enter_context(tc.tile_pool(name="x5", bufs=7))
        x4_pool = ctx.enter_context(tc.tile_pool(name="x4", bufs=7))
        o_pool = ctx.enter_context(tc.tile_pool(name="o", bufs=4))
        o4_pool = ctx.enter_context(tc.tile_pool(name="o4", bufs=4))
        ps_pool = ctx.enter_context(tc.tile_pool(name="ps", bufs=2, space="PSUM"))

        blob_t = consts.tile([128, BLOB_W], f16, tag="blob")
        nc.sync.dma_start(out=blob_t[:], in_=blob_d)
        s_t = consts.tile([97, FD], f16, tag="s")
        nc.gpsimd.dma_start(out=s_t[0:64], in_=s_d[0:64])
        nc.gpsimd.dma_start(out=s_t[64:96], in_=s_d[64:96])
        nc.gpsimd.dma_start(out=s_t[96:97], in_=s_d[96:97])
        scl_t = blob_t[:, 1216:1216 + 2 * NT].bitcast(f32)

        # ---- PE warmup: dense matmuls so the HAM clock gate flips to
        # 2.4 GHz before the first real matmul; results are dead writes.
        wu = consts.tile([128, 512], f16, tag="wu")
        nc.vector.memset(wu[:], 0.0)
        ps_w = ps_pool.tile([128, 2048], f32, tag="ps", name="ps_warm")
        for i in range(10):
            nc.tensor.matmul(
                ps_w[:, 0:512], wu[:, 0:128], wu[:],
                start=True, stop=True, skip_group_check=True,
            )

        # process the fiddly 96-row tail slot mid-stream (its small split
        # DMAs ride the DMA-bound middle's slack) so the kernel ends on
        # full-width tiles, and the last tile's stores fan out across the
        # by-then idle HWDGE queues.
        order = [0, 1, 4, 2, 3] if NT == 5 else list(range(NT))
        for t in order:
            r0, R = TILES[t]
            last = t == order[-1]
            diagT = blob_t[0:R, 608 + r0:608 + r0 + R]
            lsT = blob_t[0:97, r0:r0 + R]
            w0 = scl_t[0:R, t:t + 1]
            for half in range(2):
                c0 = HW_ * half
                x5h = x5_pool.tile([R, HW_], f16, tag="x5", name=f"x5_{t}_{half}")
                x4h = x4_pool.tile([R, HW_], f16, tag="x4", name=f"x4_{t}_{half}")
                nc.sync.dma_start(out=x5h[:], in_=x5_d[r0:r0 + R, c0:c0 + HW_])
                nc.scalar.dma_start(out=x4h[:], in_=x4_d[r0:r0 + R, c0:c0 + HW_])
                oh = (o_pool.tile([R, HW_], f16, tag="o", name=f"o_{t}_{half}")
                      if R == 128 else None)

                for sub in range(2):
                    lc = 2048 * sub           # local col in half tile
                    gc = c0 + lc              # global col
                    ps = ps_pool.tile([R, 2048], f32, tag="ps",
                                      name=f"ps_{t}_{half}_{sub}")
                    for h in range(4):
                        nc.tensor.matmul(
                            ps[:, 512 * h:512 * h + 512], diagT,
                            x4h[:, lc + 512 * h:lc + 512 * h + 512],
                            start=True, stop=False, skip_group_check=True,
                        )
                    for h in range(4):
                        nc.tensor.matmul(
                            ps[:, 512 * h:512 * h + 512], lsT,
                            s_t[0:97, gc + 512 * h:gc + 512 * h + 512],
                            start=False, stop=True, skip_group_check=True,
                        )
                    if oh is not None:
                        nc.vector.scalar_tensor_tensor(
                            oh[:, lc:lc + 2048], x5h[:, lc:lc + 2048], w0,
                            ps[:], mult, add)
                    else:
                        o4 = o4_pool.tile([R, 2048], f16, tag="o4",
                                          name=f"o4_{half}_{sub}")
                        nc.vector.scalar_tensor_tensor(
                            o4[:], x5h[:, lc:lc + 2048], w0, ps[:], mult, add)
                        qe = nc.scalar if sub == 0 else nc.gpsimd
                        qe.dma_start(
                            out=out_d[r0:r0 + R, gc:gc + 2048], in_=o4[:])
                if oh is not None:
                    if last:
                        nc.sync.dma_start(
                            out=out_d[r0:r0 + 64, c0:c0 + HW_], in_=oh[0:64])
                        nc.scalar.dma_start(
                            out=out_d[r0 + 64:r0 + R, c0:c0 + HW_],
                            in_=oh[64:R])
                    else:
                        nc.gpsimd.dma_start(out=out_d[r0:r0 + R, c0:c0 + HW_],
                                            in_=oh[:])

    nc.compile()
    return nc


def _get_program():
    if "nc" not in _cache:
        _cache["nc"] = _build_program()
    return _cache["nc"]


def _rowmap(core):
    """slot row p -> physical row, rotating the four 128-row tiles by core."""
    m = np.empty(ROWS, dtype=np.int64)
    for s in range(4):
        tp = (s + core) % 4
        m[128 * s:128 * s + 128] = np.arange(128 * tp, 128 * tp + 128)
    m[512:608] = np.arange(512, 608)
    return m


def _blob_for(w, b, rowmap):
    """Per-core const blob in slot coordinates."""
    q = rowmap                      # physical row for each slot row p
    kk = q % K
    gg = q // K
    pp = np.arange(ROWS)
    blob = np.zeros((128, BLOB_W), dtype=np.float16)
    # LS[32*s + g(q), p] = w[k(q), 2+s]; LS[96, p] = bias[k(q)]
    for s in range(3):
        blob[32 * s + gg, pp] = w[kk, 2 + s].astype(np.float16)
    blob[96, pp] = b[kk].astype(np.float16)
    # GD[p % 128, 608 + p] = w1[k(q)] (diagonal in slot coords)
    blob[pp % 128, 608 + pp] = w[kk, 1].astype(np.float16)
    # w0 per-partition f32 scalars per slot, raw-bitcast into fp16 cols
    scl = np.zeros((128, NT), dtype=np.float32)
    for t, (r0, R) in enumerate(TILES):
        scl[:R, t] = w[kk[r0:r0 + R], 0]
    blob[:, 1216:1216 + 2 * NT] = scl.view(np.float16)
    return blob


def _pack_kchw(a16):
    """[K, CH, FD] fp16 -> [ROWS, FD], row = 19*g + k."""
    return np.ascontiguousarray(a16.transpose(1, 0, 2)).reshape(ROWS, FD)


def run(inputs, trace=False, tmpdir=None):
    from concourse.bass_utils import run_bass_kernel_spmd

    w = np.asarray(inputs["weight"], dtype=np.float32)
    b = np.asarray(inputs["bias"], dtype=np.float32)
    nc = _get_program()

    s1h = np.asarray(inputs["side1"]).astype(np.float16).reshape(B, CH, FD)
    s2h = np.asarray(inputs["side2"]).astype(np.float16).reshape(B, CH, FD)
    s3h = np.asarray(inputs["side3"]).astype(np.float16).reshape(B, CH, FD)
    s4h = np.asarray(inputs["side4"]).astype(np.float16).reshape(B, K, CH, FD)
    s5h = np.asarray(inputs["side5"]).astype(np.float16).reshape(B, K, CH, FD)

    in_maps = []
    rowmaps = []
    for c in range(N_CORES):
        sp = np.empty((97, FD), dtype=np.float16)
        sp[0:32] = s1h[c]
        sp[32:64] = s2h[c]
        sp[64:96] = s3h[c]
        sp[96] = np.float16(1.0)
        rm = _rowmap(c)
        rowmaps.append(rm)
        # cores 4-7 additionally swap the two column halves of every row
        # (and of the singles, so matmul columns stay consistent) to break
        # the remaining lockstep pairing with cores 0-3; the swap is its
        # own inverse and is undone on the output below.
        if c >= 4:
            sp[0:96] = np.concatenate([sp[0:96, HW_:], sp[0:96, :HW_]], axis=1)
            x5p = _pack_kchw(s5h[c])[rm]
            x4p = _pack_kchw(s4h[c])[rm]
            x5p = np.concatenate([x5p[:, HW_:], x5p[:, :HW_]], axis=1)
            x4p = np.concatenate([x4p[:, HW_:], x4p[:, :HW_]], axis=1)
        else:
            x5p = _pack_kchw(s5h[c])[rm]
            x4p = _pack_kchw(s4h[c])[rm]
        in_maps.append({
            "x5": x5p,
            "x4": x4p,
            "s": sp,
            "blob": _blob_for(w, b, rm),
        })

    res = run_bass_kernel_spmd(nc, in_maps, list(range(N_CORES)),
                               trace=trace, tmpdir=tmpdir)
    outs = []
    for c in range(N_CORES):
        o_slot = res.results[c]["out"]
        if c >= 4:
            o_slot = np.concatenate([o_slot[:, HW_:], o_slot[:, :HW_]], axis=1)
        o_phys = np.empty_like(o_slot)
        o_phys[rowmaps[c]] = o_slot
        o = o_phys.reshape(CH, K, FD).transpose(1, 0, 2)
        outs.append(o.reshape(1, K, H, W).astype(np.float32))
    return np.concatenate(outs, axis=0), res


def kernel(**inputs):
    out, _ = run(inputs, trace=False)
    return out


# revision 47
# speedup vs baseline: 1.1940x; 1.1507x over previous
"""Trainium2 Bass kernel for nn_GroupedConvFuseSide4.

out[b,k] = w[k,0]*side5[b,k] + w[k,1]*side4[b,k]
         + w[k,2]*side1[b,0] + w[k,3]*side2[b,0] + w[k,4]*side3[b,0] + bias[k]

Sharding: pure data parallel over batch (B=8) across 8 NeuronCores.

Final scheme — fp16 wire, half-tile streaming, per-core phase rotation:
  Pixels of one image are split into CH=32 chunks of FD=8192.  Row
  r = 19*g + k gives ROWS=608 rows; 5 row-tiles (4x128 + 96) x 2
  column halves stream through SBUF as [R, 4096] 1 MB transfers with
  7-deep pools, so the free->load->use chain never starves compute.
  All 8 NeuronCores share one device's ~2.9 TB/s HBM and run the same
  SPMD program in lockstep; the chip-level floor for 8 x ~31.8 MB of
  wire traffic is ~88 us + ~7 us engine preamble, plus a ~20 us
  reduced-rate DMA phase at kernel start that persists regardless of
  data layout (the shared program reads fixed address ranges, so
  host-side permutation cannot decorrelate the cores' HBM access
  phase; true decorrelation would need partition-id-driven register
  DRAM offsets).  Core c nonetheless processes the four full
  row-tiles in a rotated order via host row-permutation of x5/x4 and
  a per-core const blob (bit-exact, zero device cost; measured
  neutral-to-positive), with the output un-permuted on host.
  The three singles + a ones row are RESIDENT: one [97, FD] SBUF tile
  (row 32*s+g, row 96 = ones for bias) loaded once; the rhs column of
  the singles matmul is the pixel position within a chunk, so the
  rotation does not touch it.  Per 512-col region:
    - PE: diag(w1) @ x4  (start=True) + lsT @ [singles; ones]
      (stop=True).  Both stationary operands are 128-col slices of ONE
      per-core [128, 1232] fp16 const blob (LS | GD | f32-scalars
      bitcast), a single ~2.4 KB/partition DMA.  All 4 diag matmuls
      run back-to-back, then all 4 ls matmuls, so the PE pays 2 weight
      swaps per block instead of 8 (a swap costs ~170ns of refill).
    - DVE: one STT per [R, 2048] block: out = x5*w0 + psum (measured
      2.34us/block; a single full pass is the DVE minimum since STT
      never gets the 2x packed mode).
  A 10-matmul warmup burst runs at kernel start so the PE HAM clock
  gate flips to 2.4 GHz before the real matmuls begin.
  Cores 4-7 additionally swap the two column halves of every row (and
  of the singles) so all 8 cores have distinct (rotation, half) phase
  offsets.  DMA queues balanced ~10.5 MB each: sync = blob + all x5
  loads, scalar = all x4 loads + last-tile [0:64] stores, gpsimd =
  singles + half-tile 1 MB stores + tail-tile block stores +
  last-tile [64:128] stores.  Loads are never queued behind stores
  on any queue.
  (Direct [96, N] DMAs measured FASTER than 64+32 splits — the v3-era
  "counts >64 and !=128 are degenerate" rule did not reproduce.)
  Host converts to fp16 and repacks so every load is contiguous;
  output comes back fp16, upcast on host.
  Max rel err vs the f32 reference ~8e-4, well under the 2e-2 gate.
"""

import numpy as np

B, K, H, W = 8, 19, 512, 512
FD = 8192                  # pixels per chunk
CH = 32                    # chunks per image (H*W / FD)
ROWS = K * CH              # 608 packed rows per core
TILES = []                 # slot (row0, nrows): 4 x 128 + 1 x 96
_r = 0
while _r < ROWS:
    TILES.append((_r, min(128, ROWS - _r)))
    _r += 128
NT = len(TILES)
HW_ = FD // 2              # half-tile columns (4096)
N_CORES = 8
BLOB_W = 1232              # 608 LS + 608 GD + 10 scl-f32-raw + 6 pad

_cache = {}


def _build_program():
    import concourse.bacc as bacc
    import concourse.tile as tile
    import concourse.mybir as mybir
    from contextlib import ExitStack

    f16 = mybir.dt.float16
    f32 = mybir.dt.float32
    mult = mybir.AluOpType.mult
    add = mybir.AluOpType.add

    nc = bacc.Bacc(
        "TRN2", target_bir_lowering=False, debug=False,
        enable_asserts=False, num_devices=N_CORES,
        enable_partition_id=False,
    )

    x5_d = nc.dram_tensor("x5", [ROWS, FD], f16, kind="ExternalInput").ap()
    x4_d = nc.dram_tensor("x4", [ROWS, FD], f16, kind="ExternalInput").ap()
    s_d = nc.dram_tensor("s", [97, FD], f16, kind="ExternalInput").ap()
    blob_d = nc.dram_tensor("blob", [128, BLOB_W], f16,
                            kind="ExternalInput").ap()
    out_d = nc.dram_tensor("out", [ROWS, FD], f16, kind="ExternalOutput").ap()

    with tile.TileContext(nc) as tc, ExitStack() as ctx:
        consts = ctx.enter_context(tc.tile_pool(name="consts", bufs=1))
        x5_pool = ctx.enter_context(tc.tile_pool(name="x5", bufs=8))
        x4_pool = ctx.enter_context(tc.tile_pool(name="x4", bufs=8))
        o_pool = ctx.enter_context(tc.tile_pool(name="o", bufs=4))
        o4_pool = ctx.enter_context(tc.tile_pool(name="o4", bufs=4))
        ps_pool = ctx.enter_context(tc.tile_pool(name="ps", bufs=2, space="PSUM"))

        blob_t = consts.tile([128, BLOB_W], f16, tag="blob")
        nc.sync.dma_start(out=blob_t[:], in_=blob_d)
        s_t = consts.tile([97, FD], f16, tag="s")
        nc.gpsimd.dma_start(out=s_t[0:64], in_=s_d[0:64])
        nc.gpsimd.dma_start(out=s_t[64:96], in_=s_d[64:96])
        nc.gpsimd.dma_start(out=s_t[96:97], in_=s_d[96:97])
        scl_t = blob_t[:, 1216:1216 + 2 * NT].bitcast(f32)

        # ---- PE warmup: dense matmuls so the HAM clock gate flips to
        # 2.4 GHz before the first real matmul; results are dead writes.
        wu = consts.tile([128, 512], f16, tag="wu")
        nc.vector.memset(wu[:], 0.0)
        ps_w = ps_pool.tile([128, 2048], f32, tag="ps", name="ps_warm")
        for i in range(10):
            nc.tensor.matmul(
                ps_w[:, 0:512], wu[:, 0:128], wu[:],
                start=True, stop=True, skip_group_check=True,
            )

        # process the fiddly 96-row tail slot mid-stream (its small split
        # DMAs ride the DMA-bound middle's slack) so the kernel ends on
        # full-width tiles, and the last tile's stores fan out across the
        # by-then idle HWDGE queues.
        order = [0, 1, 4, 2, 3] if NT == 5 else list(range(NT))
        for t in order:
            r0, R = TILES[t]
            last = t == order[-1]
            diagT = blob_t[0:R, 608 + r0:608 + r0 + R]
            lsT = blob_t[0:97, r0:r0 + R]
            w0 = scl_t[0:R, t:t + 1]
            for half in range(2):
                c0 = HW_ * half
                x5h = x5_pool.tile([R, HW_], f16, tag="x5", name=f"x5_{t}_{half}")
                x4h = x4_pool.tile([R, HW_], f16, tag="x4", name=f"x4_{t}_{half}")
                nc.sync.dma_start(out=x5h[:], in_=x5_d[r0:r0 + R, c0:c0 + HW_])
                nc.scalar.dma_start(out=x4h[:], in_=x4_d[r0:r0 + R, c0:c0 + HW_])
                oh = (o_pool.tile([R, HW_], f16, tag="o", name=f"o_{t}_{half}")
                      if R == 128 else None)

                for sub in range(2):
                    lc = 2048 * sub           # local col in half tile
                    gc = c0 + lc              # global col
                    ps = ps_pool.tile([R, 2048], f32, tag="ps",
                                      name=f"ps_{t}_{half}_{sub}")
                    for h in range(4):
                        nc.tensor.matmul(
                            ps[:, 512 * h:512 * h + 512], diagT,
                            x4h[:, lc + 512 * h:lc + 512 * h + 512],
                            start=True, stop=False, skip_group_check=True,
                        )
                    for h in range(4):
                        nc.tensor.matmul(
                            ps[:, 512 * h:512 * h + 512], lsT,
                            s_t[0:97, gc + 512 * h:gc + 512 * h + 512],
                            start=False, stop=True, skip_group_check=True,
                        )
                    if oh is not None:
                        nc.vector.scalar_tensor_tensor(
                            oh[:, lc:lc + 2048], x5h[:, lc:lc + 2048], w0,
                            ps[:], mult, add)
                        if last:
                            # drain the final tile block-by-block so the very
                            # last store dependency is only 0.5 MB.  Half 0's
                            # stores ride gpsimd (store-only queue) so they
                            # never sit ahead of half 1's load triggers in
                            # the sync/scalar instruction streams; half 1's
                            # stores fan out across the by-then idle HWDGE
                            # queues.
                            if half == 0:
                                qe = nc.gpsimd
                            else:
                                qe = nc.scalar if sub == 0 else nc.sync
                            qe.dma_start(out=out_d[r0:r0 + R, gc:gc + 2048],
                                         in_=oh[:, lc:lc + 2048])
                    else:
                        o4 = o4_pool.tile([R, 2048], f16, tag="o4",
                                          name=f"o4_{half}_{sub}")
                        nc.vector.scalar_tensor_tensor(
                            o4[:], x5h[:, lc:lc + 2048], w0, ps[:], mult, add)
                        nc.gpsimd.dma_start(
                            out=out_d[r0:r0 + R, gc:gc + 2048], in_=o4[:])
                if oh is not None and not last:
                    nc.gpsimd.dma_start(out=out_d[r0:r0 + R, c0:c0 + HW_],
                                        in_=oh[:])

    nc.compile()
    return nc


def _get_program():
    if "nc" not in _cache:
        _cache["nc"] = _build_program()
    return _cache["nc"]


def _rowmap(core):
    """slot row p -> physical row, rotating the four 128-row tiles by core."""
    m = np.empty(ROWS, dtype=np.int64)
    for s in range(4):
        tp = (s + core) % 4
        m[128 * s:128 * s + 128] = np.arange(128 * tp, 128 * tp + 128)
    m[512:608] = np.arange(512, 608)
    return m


def _blob_for(w, b, rowmap, rot=0):
    """Per-core const blob in slot coordinates.  `rot` rotates the singles
    chunk rows (matching the np.roll of the per-core S tile) so cores read
    their S buffers at decorrelated offsets during the lockstep start."""
    q = rowmap                      # physical row for each slot row p
    kk = q % K
    gg = q // K
    pp = np.arange(ROWS)
    blob = np.zeros((128, BLOB_W), dtype=np.float16)
    # LS[32*s + (g(q)+rot)%32, p] = w[k(q), 2+s]; LS[96, p] = bias[k(q)]
    for s in range(3):
        blob[32 * s + (gg + rot) % 32, pp] = w[kk, 2 + s].astype(np.float16)
    blob[96, pp] = b[kk].astype(np.float16)
    # GD[p % 128, 608 + p] = w1[k(q)] (diagonal in slot coords)
    blob[pp % 128, 608 + pp] = w[kk, 1].astype(np.float16)
    # w0 per-partition f32 scalars per slot, raw-bitcast into fp16 cols
    scl = np.zeros((128, NT), dtype=np.float32)
    for t, (r0, R) in enumerate(TILES):
        scl[:R, t] = w[kk[r0:r0 + R], 0]
    blob[:, 1216:1216 + 2 * NT] = scl.view(np.float16)
    return blob


def _pack_kchw(a16):
    """[K, CH, FD] fp16 -> [ROWS, FD], row = 19*g + k."""
    return np.ascontiguousarray(a16.transpose(1, 0, 2)).reshape(ROWS, FD)


def run(inputs, trace=False, tmpdir=None):
    from concourse.bass_utils import run_bass_kernel_spmd

    w = np.asarray(inputs["weight"], dtype=np.float32)
    b = np.asarray(inputs["bias"], dtype=np.float32)
    nc = _get_program()

    s1h = np.asarray(inputs["side1"]).astype(np.float16).reshape(B, CH, FD)
    s2h = np.asarray(inputs["side2"]).astype(np.float16).reshape(B, CH, FD)
    s3h = np.asarray(inputs["side3"]).astype(np.float16).reshape(B, CH, FD)
    s4h = np.asarray(inputs["side4"]).astype(np.float16).reshape(B, K, CH, FD)
    s5h = np.asarray(inputs["side5"]).astype(np.float16).reshape(B, K, CH, FD)

    in_maps = []
    rowmaps = []
    for c in range(N_CORES):
        rot = (8 * c) % 32
        sp = np.empty((97, FD), dtype=np.float16)
        sp[0:32] = np.roll(s1h[c], rot, axis=0)
        sp[32:64] = np.roll(s2h[c], rot, axis=0)
        sp[64:96] = np.roll(s3h[c], rot, axis=0)
        sp[96] = np.float16(1.0)
        rm = _rowmap(c)
        rowmaps.append(rm)
        # cores 4-7 additionally swap the two column halves of every row
        # (and of the singles, so matmul columns stay consistent) to break
        # the remaining lockstep pairing with cores 0-3; the swap is its
        # own inverse and is undone on the output below.
        if c >= 4:
            sp[0:96] = np.concatenate([sp[0:96, HW_:], sp[0:96, :HW_]], axis=1)
            x5p = _pack_kchw(s5h[c])[rm]
            x4p = _pack_kchw(s4h[c])[rm]
            x5p = np.concatenate([x5p[:, HW_:], x5p[:, :HW_]], axis=1)
            x4p = np.concatenate([x4p[:, HW_:], x4p[:, :HW_]], axis=1)
        else:
            x5p = _pack_kchw(s5h[c])[rm]
            x4p = _pack_kchw(s4h[c])[rm]
        in_maps.append({
            "x5": x5p,
            "x4": x4p,
            "s": sp,
            "blob": _blob_for(w, b, rm, rot),
        })

    res = run_bass_kernel_spmd(nc, in_maps, list(range(N_CORES)),
                               trace=trace, tmpdir=tmpdir)
    outs = []
    for c in range(N_CORES):
        o_slot = res.results[c]["out"]
        if c >= 4:
            o_slot = np.concatenate([o_slot[:, HW_:], o_slot[:, :HW_]], axis=1)
        o_phys = np.empty_like(o_slot)
        o_phys[rowmaps[c]] = o_slot
        o = o_phys.reshape(CH, K, FD).transpose(1, 0, 2)
        outs.append(o.reshape(1, K, H, W).astype(np.float32))
    return np.concatenate(outs, axis=0), res


def kernel(**inputs):
    out, _ = run(inputs, trace=False)
    return out
